# revision 18
# baseline (speedup 1.0000x reference)
"""AdaptiveCLPL loss on 8 TRN2 NeuronCores (Bass/Tile).

loss = mean_b [ psi(avg_cand) + sum_head psi(-l)*(1-mask) + ts*sum_samp psi(-l)*(1-is_cand) ]
with psi(u) = log1p(exp(-u)) = softplus(-u), so psi(-l) = softplus(l).

Decomposition (only term1 is per-row nonlinear; everything else sums):
  total = sum_b softplus(-avg_b)
        + [sum_{head block} softplus(l)   - sum_k uniq*inhead*softplus(l_cand)]
        + ts*[sum_{sampled rows} softplus(l) - sum_k uniq*mult*softplus(l_cand)]
  where uniq  = first-occurrence flag among the row's K candidates,
        mult  = multiplicity of the candidate column in the sampled list,
        inhead= candidate column < HEAD.

Data layout: each core gets the TRANSPOSED batch shard lT = logits[rows].T
([C, RB], row-major -> one class per row, 1KB contiguous). Head block =
rows [0, HEAD) read contiguously; sampled rows via one indirect row-gather
(100 descriptors x 1KB); candidate logits via one indirect flat gather
(2560 x 4B descriptors, index = c*RB + b).
"""

import os
import numpy as np

B, C, K = 2048, 50000, 10
HEAD, S = 2000, 100
TSCALE = float(C - HEAD) / float(S)  # 480.0
NCORES = 8
RB = B // NCORES  # 256 rows per core
P = 128           # cand tiles: partitions = row b%128, col groups of K per half
HGROUPS = 4       # head split for DMA/ACT pipelining
HROWS = HEAD // HGROUPS   # 500 rows per group
HP = 125                  # partitions per head tile
HB = HROWS // HP          # 4 blocks -> free dim 1024

_CACHE = {}


def _build(native_softplus=True, enable_asserts=False, stage=99):
    import concourse.bass as bass
    import concourse.tile as tile
    from concourse import bacc, bass_isa, mybir

    f32 = mybir.dt.float32
    i32 = mybir.dt.int32
    AF = mybir.ActivationFunctionType
    OP = mybir.AluOpType
    AX = mybir.AxisListType

    nc = bacc.Bacc(
        "TRN2",
        target_bir_lowering=False,
        debug=False,
        enable_asserts=enable_asserts,
        num_devices=NCORES,
    )

    lT = nc.dram_tensor("lT", [C, RB], f32, kind="ExternalInput").ap()
    gidx = nc.dram_tensor("gidx", [P, 2 * K], i32, kind="ExternalInput").ap()
    candf = nc.dram_tensor("candf", [P, 2 * K], f32, kind="ExternalInput").ap()
    sidx = nc.dram_tensor("sidx", [S, 1], i32, kind="ExternalInput").ap()
    targ = nc.dram_tensor("targ", [P, S], f32, kind="ExternalInput").ap()
    tri = nc.dram_tensor("tri", [P, K * K], f32, kind="ExternalInput").ap()
    out = nc.dram_tensor("out", [1, 1], f32, kind="ExternalOutput").ap()

    def softplus(out_ap, in_ap, scale=1.0, accum=None):
        """out = softplus(in_*scale); accum (optional) = row sums of out."""
        if native_softplus:
            nc.scalar.activation(out_ap, in_ap, AF.Softplus, scale=scale,
                                 accum_out=accum)
        else:
            nc.scalar.activation(out_ap, in_ap, AF.Exp, scale=scale)
            nc.scalar.activation(out_ap, out_ap, AF.Ln, bias=1.0,
                                 accum_out=accum)

    with tile.TileContext(nc) as tc:
        with tc.tile_pool(name="sb", bufs=1) as sb:
            total = sb.tile([P, 1], f32)
            nc.vector.memset(total[:, :], 0.0)
            # ---------------- head term ----------------
            if stage >= 1:
                hacc = sb.tile([HP, HGROUPS], f32)
                for g in range(HGROUPS):
                    ht = sb.tile([HP, HB * RB], f32, tag="ht", bufs=2)
                    src = lT[g * HROWS:(g + 1) * HROWS, :].rearrange(
                        "(p j) c -> p (j c)", j=HB)
                    nc.sync.dma_start(out=ht[:, :], in_=src)
                    hs = sb.tile([HP, HB * RB], f32, tag="hs", bufs=2)
                    softplus(hs[:, :], ht[:, :], accum=hacc[:, g:g + 1])
                hred = sb.tile([HP, 1], f32)
                nc.vector.tensor_reduce(hred[:, :], hacc[:, :], AX.X, OP.add)
                nc.vector.tensor_tensor(total[:HP, :], total[:HP, :],
                                        hred[:, :], op=OP.add)

            # ---------------- sampled-tail term ----------------
            if stage >= 2:
                sidx_t = sb.tile([S, 1], i32)
                nc.sync.dma_start(out=sidx_t[:, :], in_=sidx[:, :])
                samp = sb.tile([S, RB], f32)
                nc.gpsimd.indirect_dma_start(
                    out=samp[:, :],
                    out_offset=None,
                    in_=lT[:, :],
                    in_offset=bass.IndirectOffsetOnAxis(ap=sidx_t[:, :1],
                                                        axis=0),
                )
                ssp = sb.tile([S, RB], f32)
                sacc = sb.tile([S, 1], f32)
                softplus(ssp[:, :], samp[:, :], accum=sacc[:, :])
                sacc2 = sb.tile([S, 1], f32)
                nc.vector.tensor_scalar_mul(sacc2[:, :], sacc[:, :], TSCALE)
                nc.vector.tensor_tensor(total[:S, :], total[:S, :],
                                        sacc2[:, :], op=OP.add)

            # ---------------- candidate gather ----------------
            if stage >= 3:
                gidx_t = sb.tile([P, 2 * K], i32)
                nc.sync.dma_start(out=gidx_t[:, :], in_=gidx[:, :])
                candf_t = sb.tile([P, 2 * K], f32)
                nc.sync.dma_start(out=candf_t[:, :], in_=candf[:, :])
                targ_t = sb.tile([P, S], f32)
                nc.sync.dma_start(out=targ_t[:, :], in_=targ[:, :])
                tri_t = sb.tile([P, K * K], f32)
                nc.sync.dma_start(out=tri_t[:, :], in_=tri[:, :])

                # HW indirect DMA = one index per partition, contiguous block
                # per descriptor -> gather the candidate columns one at a time.
                clog = sb.tile([P, 2 * K], f32)
                for k in range(2 * K):
                    nc.gpsimd.indirect_dma_start(
                        out=clog[:, k:k + 1],
                        out_offset=None,
                        in_=lT[:, :],
                        in_offset=bass.IndirectOffsetOnAxis(
                            ap=gidx_t[:, k:k + 1], axis=1),
                    )
                spc = sb.tile([P, 2 * K], f32)
                softplus(spc[:, :], clog[:, :])

            if stage >= 4:
                self_build_cand_terms = True
            else:
                self_build_cand_terms = False

            if self_build_cand_terms:
                # uniq: zero count of equal-to-earlier within each K group
                dcnt = sb.tile([P, 2 * K], f32)
                for g in range(2):
                    sl = candf_t[:, g * K:(g + 1) * K]
                    eq = sb.tile([P, K * K], f32, tag="eq", bufs=2)
                    nc.vector.tensor_tensor(
                        out=eq[:, :].rearrange("p (a b) -> p a b", a=K),
                        in0=sl.unsqueeze(2).to_broadcast([P, K, K]),
                        in1=sl.unsqueeze(1).to_broadcast([P, K, K]),
                        op=OP.is_equal,
                    )
                    # keep only k' < k (strict lower triangle)
                    nc.vector.tensor_tensor(eq[:, :], eq[:, :], tri_t[:, :],
                                            op=OP.mult)
                    nc.vector.tensor_reduce(
                        dcnt[:, g * K:(g + 1) * K],
                        eq[:, :].rearrange("p (a b) -> p a b", a=K),
                        AX.X, OP.add,
                    )
                uniq = sb.tile([P, 2 * K], f32)
                nc.vector.tensor_scalar(uniq[:, :], dcnt[:, :], 0.0, None,
                                        op0=OP.is_equal)

                # mult: multiplicity of candidate col in sampled cols
                eqs = sb.tile([P, 2 * K * S], f32)
                nc.vector.tensor_tensor(
                    out=eqs[:, :].rearrange("p (a b) -> p a b", a=2 * K),
                    in0=candf_t[:, :].unsqueeze(2).to_broadcast([P, 2 * K, S]),
                    in1=targ_t[:, :].unsqueeze(1).to_broadcast([P, 2 * K, S]),
                    op=OP.is_equal,
                )
                mult = sb.tile([P, 2 * K], f32)
                nc.vector.tensor_reduce(
                    mult[:, :],
                    eqs[:, :].rearrange("p (a b) -> p a b", a=2 * K),
                    AX.X, OP.add,
                )

                # inhead: candidate col < HEAD
                inhead = sb.tile([P, 2 * K], f32)
                nc.vector.tensor_scalar(inhead[:, :], candf_t[:, :],
                                        float(HEAD), None, op0=OP.is_lt)

                # corr = -sum_k uniq*(inhead + ts*mult)*softplus(cand)
                wmix = sb.tile([P, 2 * K], f32)
                nc.vector.tensor_scalar(wmix[:, :], mult[:, :], TSCALE, None,
                                        op0=OP.mult)
                nc.vector.tensor_tensor(wmix[:, :], wmix[:, :], inhead[:, :],
                                        op=OP.add)
                nc.vector.tensor_tensor(wmix[:, :], wmix[:, :], uniq[:, :],
                                        op=OP.mult)
                corr = sb.tile([P, 1], f32)
                scr = sb.tile([P, 2 * K], f32)
                nc.vector.tensor_tensor(scr[:, :], wmix[:, :], spc[:, :],
                                        op=OP.mult)
                nc.vector.tensor_reduce(corr[:, :], scr[:, :], AX.X, OP.add)

                # term1 = softplus(-avg); avg = sum(uniq*clog)/sum(uniq)
                csum = sb.tile([P, 2], f32)
                scr2 = sb.tile([P, 2 * K], f32)
                nc.vector.tensor_tensor(scr2[:, :], uniq[:, :], clog[:, :],
                                        op=OP.mult)
                nc.vector.tensor_reduce(
                    csum[:, :],
                    scr2[:, :].rearrange("p (g k) -> p g k", g=2),
                    AX.X, OP.add)
                cnt = sb.tile([P, 2], f32)
                nc.vector.tensor_reduce(
                    cnt[:, :],
                    uniq[:, :].rearrange("p (g k) -> p g k", g=2),
                    AX.X, OP.add)
                rcp = sb.tile([P, 2], f32)
                nc.vector.reciprocal(rcp[:, :], cnt[:, :])
                avg = sb.tile([P, 2], f32)
                nc.vector.tensor_tensor(avg[:, :], csum[:, :], rcp[:, :],
                                        op=OP.mult)
                t1 = sb.tile([P, 2], f32)
                t1col = sb.tile([P, 1], f32)
                softplus(t1[:, :], avg[:, :], scale=-1.0, accum=t1col[:, :])

                nc.vector.tensor_tensor(total[:, :], total[:, :], t1col[:, :],
                                        op=OP.add)
                nc.vector.tensor_tensor(total[:, :], total[:, :], corr[:, :],
                                        op=OP.subtract)

            # ---------------- combine ----------------
            gtot = sb.tile([P, 1], f32)
            nc.gpsimd.partition_all_reduce(gtot[:, :], total[:, :],
                                           channels=P,
                                           reduce_op=bass_isa.ReduceOp.add)
            res = sb.tile([1, 1], f32)
            nc.vector.tensor_scalar_mul(res[:, :], gtot[0:1, :], 1.0 / B)
            nc.sync.dma_start(out=out[:, :], in_=res[:, :])

    nc.compile()
    return nc


def prep_inputs(logits, candidates, sampled_indices):
    """Full inputs -> per-core in_maps (host-side sharding/index prep only)."""
    logits = np.asarray(logits)
    candidates = np.asarray(candidates)
    sampled_indices = np.asarray(sampled_indices)
    assert logits.shape == (B, C) and candidates.shape == (B, K)
    srow = (HEAD + sampled_indices.astype(np.int64)).astype(np.int32)
    sidx = srow.reshape(S, 1)
    targ = np.broadcast_to(srow.astype(np.float32), (P, S)).copy()
    tri1 = (np.arange(K)[:, None] > np.arange(K)[None, :]).astype(np.float32)
    tri = np.broadcast_to(tri1.reshape(1, K * K), (P, K * K)).copy()
    in_maps = []
    for i in range(NCORES):
        rows = slice(i * RB, (i + 1) * RB)
        lT = np.ascontiguousarray(logits[rows].T.astype(np.float32, copy=False))
        cand = candidates[rows].astype(np.int64)
        gidx_full = (cand * RB + np.arange(RB)[:, None]).astype(np.int32)
        gidx = np.concatenate(
            [gidx_full[:P], gidx_full[P:]], axis=1)  # [128, 20]
        cf = cand.astype(np.float32)
        candf = np.concatenate([cf[:P], cf[P:]], axis=1)  # [128, 20]
        in_maps.append({
            "lT": lT,
            "gidx": np.ascontiguousarray(gidx),
            "candf": np.ascontiguousarray(candf),
            "sidx": sidx,
            "targ": targ,
            "tri": tri,
        })
    return in_maps


def get_graph(native_softplus=True, enable_asserts=False, stage=99):
    key = (native_softplus, enable_asserts, stage)
    if key not in _CACHE:
        _CACHE[key] = _build(native_softplus=native_softplus,
                             enable_asserts=enable_asserts, stage=stage)
    return _CACHE[key]


def run(logits, candidates, sampled_indices, trace=False, **kw):
    """Returns (scalar float32 loss, BassKernelResults)."""
    from concourse.bass_utils import run_bass_kernel_spmd

    native = os.environ.get("BASS_NATIVE_SOFTPLUS", "0") == "1"
    nc = get_graph(native_softplus=native)
    in_maps = prep_inputs(logits, candidates, sampled_indices)
    res = run_bass_kernel_spmd(nc, in_maps, core_ids=list(range(NCORES)),
                               trace=trace, **kw)
    partials = [r["out"].reshape(()) for r in res.results]
    loss = np.float32(np.sum(np.stack(partials), dtype=np.float64))
    return loss, res


def kernel(logits, candidates, sampled_indices):
    loss, _ = run(logits, candidates, sampled_indices, trace=False)
    return loss


# revision 19
# speedup vs baseline: 1.0462x; 1.0462x over previous
"""AdaptiveCLPL loss on 8 TRN2 NeuronCores (Bass/Tile).

loss = mean_b [ psi(avg_cand) + sum_head psi(-l)*(1-mask) + ts*sum_samp psi(-l)*(1-is_cand) ]
with psi(u) = log1p(exp(-u)) = softplus(-u), so psi(-l) = softplus(l).

Decomposition (only term1 is per-row nonlinear; everything else sums):
  total = sum_b softplus(-avg_b)
        + [sum_{head block} softplus(l)   - sum_k uniq*inhead*softplus(l_cand)]
        + ts*[sum_{sampled rows} softplus(l) - sum_k uniq*mult*softplus(l_cand)]
  where uniq  = first-occurrence flag among the row's K candidates,
        mult  = multiplicity of the candidate column in the sampled list,
        inhead= candidate column < HEAD.

Data layout: each core gets the TRANSPOSED batch shard lT = logits[rows].T
([C, RB], row-major -> one class per row, 1KB contiguous). Head block =
rows [0, HEAD) read contiguously; sampled rows via one indirect row-gather
(100 descriptors x 1KB); candidate logits via one indirect flat gather
(2560 x 4B descriptors, index = c*RB + b).
"""

import os
import numpy as np

B, C, K = 2048, 50000, 10
HEAD, S = 2000, 100
TSCALE = float(C - HEAD) / float(S)  # 480.0
NCORES = 8
RB = B // NCORES  # 256 rows per core
P = 128           # cand tiles: partitions = row b%128, col groups of K per half
HGROUPS = 4       # head split for DMA/ACT pipelining
HROWS = HEAD // HGROUPS   # 500 rows per group
HP = 125                  # partitions per head tile
HB = HROWS // HP          # 4 blocks -> free dim 1024

_CACHE = {}


def _build(native_softplus=False, enable_asserts=False, stage=99):
    """Build the single-core SPMD graph (composite softplus: Ln(Exp(x)+1)).

    Schedule-shaping notes:
    - tiny index DMAs + all indirect gathers are emitted FIRST so the gpsimd
      SWDGE work (20 candidate gathers ~1us each) starts at t~1us,
    - bulk Exps are batched before bulk Lns (2 act-table loads, not 10),
    - a dummy Exp re-loads the exp table during the gather window so the
      late (post-gather) Exp ops don't pay a table load on the tail,
    - candidate mask math (index-compare only) runs on DVE during gathers.
    """
    import concourse.bass as bass
    import concourse.tile as tile
    from concourse import bacc, bass_isa, mybir
    from concourse.bass import _add_dep_helper

    f32 = mybir.dt.float32
    i32 = mybir.dt.int32
    AF = mybir.ActivationFunctionType
    OP = mybir.AluOpType
    AX = mybir.AxisListType

    nc = bacc.Bacc(
        "TRN2",
        target_bir_lowering=False,
        debug=False,
        enable_asserts=enable_asserts,
        num_devices=NCORES,
    )

    lT = nc.dram_tensor("lT", [C, RB], f32, kind="ExternalInput").ap()
    gidx = nc.dram_tensor("gidx", [P, 2 * K], i32, kind="ExternalInput").ap()
    candf = nc.dram_tensor("candf", [P, 2 * K], f32, kind="ExternalInput").ap()
    sidx = nc.dram_tensor("sidx", [S, 1], i32, kind="ExternalInput").ap()
    targ = nc.dram_tensor("targ", [P, S], f32, kind="ExternalInput").ap()
    tri = nc.dram_tensor("tri", [P, K * K], f32, kind="ExternalInput").ap()
    out = nc.dram_tensor("out", [1, 1], f32, kind="ExternalOutput").ap()

    with tile.TileContext(nc) as tc:
        with tc.tile_pool(name="sb", bufs=1) as sb:
            total = sb.tile([P, 1], f32)
            nc.vector.memset(total[:, :], 0.0)

            # ---- A: index DMAs + all indirect gathers (gpsimd) ----
            sidx_t = sb.tile([S, 1], i32)
            nc.sync.dma_start(out=sidx_t[:, :], in_=sidx[:, :])
            gidx_t = sb.tile([P, 2 * K], i32)
            nc.sync.dma_start(out=gidx_t[:, :], in_=gidx[:, :])
            candf_t = sb.tile([P, 2 * K], f32)
            nc.sync.dma_start(out=candf_t[:, :], in_=candf[:, :])
            targ_t = sb.tile([P, S], f32)
            nc.sync.dma_start(out=targ_t[:, :], in_=targ[:, :])
            tri_t = sb.tile([P, K * K], f32)
            nc.sync.dma_start(out=tri_t[:, :], in_=tri[:, :])

            samp = sb.tile([S, RB], f32)
            nc.gpsimd.indirect_dma_start(
                out=samp[:, :], out_offset=None, in_=lT[:, :],
                in_offset=bass.IndirectOffsetOnAxis(ap=sidx_t[:, :1], axis=0))
            # HW indirect DMA: one index per partition, contiguous block per
            # descriptor -> the 2*K candidate columns go one DMA at a time.
            clog = sb.tile([P, 2 * K], f32)
            for k in range(2 * K):
                nc.gpsimd.indirect_dma_start(
                    out=clog[:, k:k + 1], out_offset=None, in_=lT[:, :],
                    in_offset=bass.IndirectOffsetOnAxis(
                        ap=gidx_t[:, k:k + 1], axis=1))

            # ---- B: head DMAs ----
            hts = []
            for g in range(HGROUPS):
                ht = sb.tile([HP, HB * RB], f32, tag=f"ht{g}")
                src = lT[g * HROWS:(g + 1) * HROWS, :].rearrange(
                    "(p j) c -> p (j c)", j=HB)
                nc.sync.dma_start(out=ht[:, :], in_=src)
                hts.append(ht)

            # ---- masks on DVE (index compares only; runs during gathers) ---
            dcnt = sb.tile([P, 2 * K], f32)
            for g in range(2):
                sl = candf_t[:, g * K:(g + 1) * K]
                eq = sb.tile([P, K * K], f32, tag="eq", bufs=2)
                nc.vector.tensor_tensor(
                    out=eq[:, :].rearrange("p (a b) -> p a b", a=K),
                    in0=sl.unsqueeze(2).to_broadcast([P, K, K]),
                    in1=sl.unsqueeze(1).to_broadcast([P, K, K]),
                    op=OP.is_equal)
                nc.vector.tensor_tensor(eq[:, :], eq[:, :], tri_t[:, :],
                                        op=OP.mult)
                nc.vector.tensor_reduce(
                    dcnt[:, g * K:(g + 1) * K],
                    eq[:, :].rearrange("p (a b) -> p a b", a=K),
                    AX.X, OP.add)
            uniq = sb.tile([P, 2 * K], f32)
            nc.vector.tensor_scalar(uniq[:, :], dcnt[:, :], 0.0, None,
                                    op0=OP.is_equal)
            eqs = sb.tile([P, 2 * K * S], f32)
            nc.vector.tensor_tensor(
                out=eqs[:, :].rearrange("p (a b) -> p a b", a=2 * K),
                in0=candf_t[:, :].unsqueeze(2).to_broadcast([P, 2 * K, S]),
                in1=targ_t[:, :].unsqueeze(1).to_broadcast([P, 2 * K, S]),
                op=OP.is_equal)
            mult = sb.tile([P, 2 * K], f32)
            nc.vector.tensor_reduce(
                mult[:, :], eqs[:, :].rearrange("p (a b) -> p a b", a=2 * K),
                AX.X, OP.add)
            inhead = sb.tile([P, 2 * K], f32)
            nc.vector.tensor_scalar(inhead[:, :], candf_t[:, :], float(HEAD),
                                    None, op0=OP.is_lt)
            wmix = sb.tile([P, 2 * K], f32)
            nc.vector.tensor_scalar(wmix[:, :], mult[:, :], TSCALE, None,
                                    op0=OP.mult)
            nc.vector.tensor_tensor(wmix[:, :], wmix[:, :], inhead[:, :],
                                    op=OP.add)
            nc.vector.tensor_tensor(wmix[:, :], wmix[:, :], uniq[:, :],
                                    op=OP.mult)
            cnt = sb.tile([P, 2], f32)
            nc.vector.tensor_reduce(
                cnt[:, :], uniq[:, :].rearrange("p (g k) -> p g k", g=2),
                AX.X, OP.add)
            rcp = sb.tile([P, 2], f32)
            nc.vector.reciprocal(rcp[:, :], cnt[:, :])

            # ---- C: bulk Exps, then bulk Lns (2 table loads) ----
            exps = []
            for g in range(HGROUPS):
                exps.append(nc.scalar.activation(hts[g][:, :], hts[g][:, :],
                                                 AF.Exp))
            se = nc.scalar.activation(samp[:, :], samp[:, :], AF.Exp)
            exps.append(se)
            hacc = sb.tile([HP, HGROUPS], f32)
            for g in range(HGROUPS):
                ln = nc.scalar.activation(hts[g][:, :], hts[g][:, :], AF.Ln,
                                          bias=1.0, accum_out=hacc[:, g:g + 1])
                _add_dep_helper(ln.ins, exps[-1].ins, sync=False,
                                reason="batch bulk Exps before bulk Lns")
            sacc = sb.tile([S, 1], f32)
            ln = nc.scalar.activation(samp[:, :], samp[:, :], AF.Ln, bias=1.0,
                                      accum_out=sacc[:, :])
            _add_dep_helper(ln.ins, exps[-1].ins, sync=False,
                            reason="batch bulk Exps before bulk Lns")

            hred = sb.tile([HP, 1], f32)
            nc.vector.tensor_reduce(hred[:, :], hacc[:, :], AX.X, OP.add)
            nc.vector.tensor_tensor(total[:HP, :], total[:HP, :], hred[:, :],
                                    op=OP.add)
            sacc2 = sb.tile([S, 1], f32)
            nc.vector.tensor_scalar_mul(sacc2[:, :], sacc[:, :], TSCALE)
            tadd = nc.vector.tensor_tensor(total[:S, :], total[:S, :],
                                           sacc2[:, :], op=OP.add)

            # ---- dummy Exp: reload exp table during the gather window ----
            dummy = sb.tile([1, 1], f32)
            dex = nc.scalar.activation(dummy[:, :], total[0:1, :1], AF.Exp,
                                       scale=0.0)
            _add_dep_helper(dex.ins, tadd.ins, sync=False,
                            reason="prefetch exp table after bulk Lns")

            # ---- late phase (needs clog) ----
            csum = sb.tile([P, 2], f32)
            scr2 = sb.tile([P, 2 * K], f32)
            nc.vector.tensor_tensor(scr2[:, :], uniq[:, :], clog[:, :],
                                    op=OP.mult)
            nc.vector.tensor_reduce(
                csum[:, :], scr2[:, :].rearrange("p (g k) -> p g k", g=2),
                AX.X, OP.add)
            avg = sb.tile([P, 2], f32)
            nc.vector.tensor_tensor(avg[:, :], csum[:, :], rcp[:, :],
                                    op=OP.mult)
            ce = sb.tile([P, 2 * K], f32)
            e1 = nc.scalar.activation(ce[:, :], clog[:, :], AF.Exp)
            ae = sb.tile([P, 2], f32)
            e2 = nc.scalar.activation(ae[:, :], avg[:, :], AF.Exp, scale=-1.0)
            spl = sb.tile([P, 2 * K], f32)
            l1 = nc.scalar.activation(spl[:, :], ce[:, :], AF.Ln, bias=1.0)
            _add_dep_helper(l1.ins, e2.ins, sync=False,
                            reason="batch late Exps before late Lns")
            t1 = sb.tile([P, 2], f32)
            t1col = sb.tile([P, 1], f32)
            nc.scalar.activation(t1[:, :], ae[:, :], AF.Ln, bias=1.0,
                                 accum_out=t1col[:, :])

            corr = sb.tile([P, 1], f32)
            scr = sb.tile([P, 2 * K], f32)
            nc.vector.tensor_tensor(scr[:, :], wmix[:, :], spl[:, :],
                                    op=OP.mult)
            nc.vector.tensor_reduce(corr[:, :], scr[:, :], AX.X, OP.add)

            nc.vector.tensor_tensor(total[:, :], total[:, :], t1col[:, :],
                                    op=OP.add)
            nc.vector.tensor_tensor(total[:, :], total[:, :], corr[:, :],
                                    op=OP.subtract)
            gtot = sb.tile([P, 1], f32)
            nc.gpsimd.partition_all_reduce(gtot[:, :], total[:, :],
                                           channels=P,
                                           reduce_op=bass_isa.ReduceOp.add)
            res = sb.tile([1, 1], f32)
            nc.vector.tensor_scalar_mul(res[:, :], gtot[0:1, :], 1.0 / B)
            nc.sync.dma_start(out=out[:, :], in_=res[:, :])

    nc.compile()
    return nc


def prep_inputs(logits, candidates, sampled_indices):
    """Full inputs -> per-core in_maps (host-side sharding/index prep only)."""
    logits = np.asarray(logits)
    candidates = np.asarray(candidates)
    sampled_indices = np.asarray(sampled_indices)
    assert logits.shape == (B, C) and candidates.shape == (B, K)
    srow = (HEAD + sampled_indices.astype(np.int64)).astype(np.int32)
    sidx = srow.reshape(S, 1)
    targ = np.broadcast_to(srow.astype(np.float32), (P, S)).copy()
    tri1 = (np.arange(K)[:, None] > np.arange(K)[None, :]).astype(np.float32)
    tri = np.broadcast_to(tri1.reshape(1, K * K), (P, K * K)).copy()
    in_maps = []
    for i in range(NCORES):
        rows = slice(i * RB, (i + 1) * RB)
        lT = np.ascontiguousarray(logits[rows].T.astype(np.float32, copy=False))
        cand = candidates[rows].astype(np.int64)
        gidx_full = (cand * RB + np.arange(RB)[:, None]).astype(np.int32)
        gidx = np.concatenate(
            [gidx_full[:P], gidx_full[P:]], axis=1)  # [128, 20]
        cf = cand.astype(np.float32)
        candf = np.concatenate([cf[:P], cf[P:]], axis=1)  # [128, 20]
        in_maps.append({
            "lT": lT,
            "gidx": np.ascontiguousarray(gidx),
            "candf": np.ascontiguousarray(candf),
            "sidx": sidx,
            "targ": targ,
            "tri": tri,
        })
    return in_maps


def get_graph(native_softplus=True, enable_asserts=False, stage=99):
    key = (native_softplus, enable_asserts, stage)
    if key not in _CACHE:
        _CACHE[key] = _build(native_softplus=native_softplus,
                             enable_asserts=enable_asserts, stage=stage)
    return _CACHE[key]


def run(logits, candidates, sampled_indices, trace=False, **kw):
    """Returns (scalar float32 loss, BassKernelResults)."""
    from concourse.bass_utils import run_bass_kernel_spmd

    native = os.environ.get("BASS_NATIVE_SOFTPLUS", "0") == "1"
    nc = get_graph(native_softplus=native)
    in_maps = prep_inputs(logits, candidates, sampled_indices)
    res = run_bass_kernel_spmd(nc, in_maps, core_ids=list(range(NCORES)),
                               trace=trace, **kw)
    partials = [r["out"].reshape(()) for r in res.results]
    loss = np.float32(np.sum(np.stack(partials), dtype=np.float64))
    return loss, res


def kernel(logits, candidates, sampled_indices):
    loss, _ = run(logits, candidates, sampled_indices, trace=False)
    return loss
